# revision 1
# baseline (speedup 1.0000x reference)
"""Distributed TRN2 kernel for nn_Att_scores (attention score double-sum).

Math: the reference computes qkv = X @ W_qkv.T, splits q/k, and takes
scores = (q k^T * scale).sum(heads).sum(keys).  The head/key sums commute
with the matmuls, so exactly:
    Xsum[b]      = sum_n X[b, n, :]                      # [C]
    u[b]         = Wq^T (Wk Xsum[b])                     # [C]
    scores[b, n] = scale * X[b, n, :] . u[b]

Distribution (8 cores): X row-sharded (256 rows/batch per core); W split in
HALVES across SEngine pairs (core i holds rows h*384:(h+1)*384 of Wq and
Wk, h = i%2), so u = u_0 + u_1 with u_h = Wq_h^T (Wk_h Xsum).

Inputs are pre-cast to bf16 host-side in make_in_maps (the device matmuls
run bf16 anyway; halves HBM traffic to ~2MB/core).  Loads ride HWDGE
(nc.sync) as plain bf16 moves, X in 4 chunk-DMAs that pipeline with the
Xsum matmuls.

Layout discipline — every intermediate is computed directly in its
consumer's layout; there are no transposes between compute stages:
  * partial Xsum^T lands [c-part, (ck b)] via 24 matmuls with the X tiles
    stationary and a ones-vector streaming; one DVE copy feeds the DRAM
    bounce and the 6KB AllReduce runs in this transposed layout (the CC
    also serves as the global barrier for the pair exchange).
  * t_h[j, b] via lhsT = Wk_h^T 128-col slices (built by PE transposes in
    the AllReduce shadow, where engine time is free) x rhs = Xsum^T.
  * u_h^T[c', b] via lhsT = Wq_h 128-col slices x rhs = t_h, accumulated
    over the 3 j-tiles per ck chunk, written straight into the fp32 send
    buffer (remote_dma payloads must stay >= 48B/partition; bf16 payloads
    corrupt).
  * the pair exchange is one remote_dma_broadcast frame to the SEngine
    partner (XOR dtpb=1) on SWDGE queue 1 (own queue so the untriggered
    prep cannot stall the bulk loads), prepped in the load shadow and
    triggered when u^T is drained.
  * the final dot is a split 12-matmul PSUM accumulation: my u-half's 6
    matmuls run while the partner's half is in flight; the partner's 6 are
    gated by a bf16 cast carrying the attached remote-sem wait (waits are
    attached post-scheduling to the consumer's sync_info — standalone
    injected waits get merged by the legalizer and deadlock cross-core).
  * scale folds into the PSUM drains; X^T for the final dot is built by 24
    PE transposes in the AllReduce shadow.
End-of-NEFF quiesce waits drain the exchange semaphores so no descriptor
is in flight across executions.  Measured absmax relative error vs the
fp32 reference: 4.8e-3 (gate 2e-2).
"""

import numpy as np

B = 2
N = 2048
C = 768
H = 12
HD = C // H
SCALE = float(HD) ** -0.5
NCORES = 8
NS = N // NCORES          # 256 rows of each batch per core
CH = NS // 128            # 2 partition-chunks per batch per core
JT = C // 128             # 6 128-row tiles of full C
JH = JT // 2              # 3 tiles per W half

_compiled_nc = None


def _attach_wait(target_ins, sem, val):
    """Attach a HW-only semaphore wait directly to target_ins's dispatch
    conditions.  A standalone injected EventSemaphore gets merged with
    same-sem waits by the post-schedule legalizer (valid in its model where
    remote sems never advance, a cross-core deadlock for us); an extra
    SyncWait on the consumer itself is left alone.  Invisible to the Tile
    scheduling sim because it is added after scheduling."""
    import concourse.mybir as mb

    w = mb.SyncWait(
        sync_type="semaphore",
        id=sem.num,
        ant_name=sem.name,
        wait_mode="sem-ge-imm",
        wait_value=val,
    )
    si = target_ins.sync_info
    if si is None:
        target_ins.sync_info = mb.SyncInfo(on_wait=[w], on_update=[])
    else:
        si.on_wait.append(w)


def _build_and_compile(use_collective=True, repeats=1):
    import concourse.bass as bass  # noqa: F401
    import concourse.bacc as bacc
    import concourse.tile as tile
    import concourse.mybir as mybir
    from concourse import masks

    f32 = mybir.dt.float32
    bf16 = mybir.dt.bfloat16
    add = mybir.AluOpType.add
    nc = bacc.Bacc(
        "TRN2",
        target_bir_lowering=False,
        debug=False,
        num_devices=NCORES,
        num_swdge_queues=2,
    )

    x_d = nc.dram_tensor("x_in", [B, NS, C], bf16, kind="ExternalInput")
    # per-core W half: rows 0:384 = Wq_h, rows 384:768 = Wk_h (bf16, pre-cast)
    w_d = nc.dram_tensor("w_in", [C, C], bf16, kind="ExternalInput")
    out_d = nc.dram_tensor("scores_out", [B, NS], f32, kind="ExternalOutput")

    SLICES = ((0, 512), (512, 256))

    rsem = nc.alloc_semaphore("u_rsem")
    lsem = nc.alloc_semaphore("u_lsem")
    inject_specs = []  # (engine, sem, val, target_ins)

    with tile.TileContext(nc) as tc:
        with (
            tc.tile_pool(name="sbuf", bufs=1) as pool,
            tc.tile_pool(name="psum", bufs=1, space="PSUM") as psum,
            tc.tile_pool(name="dram", bufs=1, space="DRAM") as dram,
        ):
            x_bf = pool.tile([128, B * CH, C], bf16)   # [p, (b,ch), c]
            xT_sb = pool.tile([128, JT, B * NS], bf16)  # X^T: [c-part, ck, (b n)]
            wq_sb = pool.tile([128, JH, C], bf16)      # Wq half, row j=jt*128+p
            wk_sb = pool.tile([128, JH, C], bf16)      # Wk half natural
            wkT_sb = pool.tile([128, JT, 384], bf16)   # Wk_h^T: [c-part, ck, j]
            ones_red = pool.tile([128, 1], bf16)       # lhsT for row-sum
            ident_bf = pool.tile([128, 128], bf16)
            xspT_sb = pool.tile([128, JT * B], f32)    # partial Xsum^T
            xsumTf = pool.tile([128, JT * B], f32)     # reduced Xsum^T fp32
            ident_f = pool.tile([B, B], f32)
            xsumT_sb = pool.tile([128, JT, B], bf16)   # Xsum^T
            t2_sb = pool.tile([B, 384], bf16)          # t_h^T: [b, j-half]
            t_sb = pool.tile([128, JH, B], bf16)       # t_h: [j-part, jt, b]
            u2f_sb = pool.tile([B, C], f32)            # u_h fp32, b on parts
            uT_send = pool.tile([128, 2, JT * B], f32)  # parity-buffered send
            uT_recv = pool.tile([128, 2, JT * B], f32)  # partner's u half
            uTs_bf = pool.tile([128, JT * B], bf16)    # my u^T half, bf16
            uTr_bf = pool.tile([128, JT * B], bf16)    # partner's half, bf16
            out_row2 = pool.tile([B, B * NS], f32)

            for _rep in range(repeats):
                if _rep == 0:
                    nc.gpsimd.memset(ones_red[:], 1.0)
                    masks.make_identity(nc, ident_bf[:])
                    masks.make_identity(nc, ident_f[:])

                # ------- loads (HWDGE, plain bf16 — inputs pre-cast on host) -----
                for b in range(B):
                    for ch in range(CH):
                        nc.sync.dma_start(
                            x_bf[:, b * CH + ch, :],
                            x_d[b, ch * 128 : (ch + 1) * 128, :],
                        )
                nc.sync.dma_start(
                    wk_sb[:], w_d[384:768, :].rearrange("(t p) c -> p t c", p=128)
                )
                nc.sync.dma_start(
                    wq_sb[:], w_d[0:384, :].rearrange("(t p) c -> p t c", p=128)
                )

                # u-exchange prep: data-independent, descgen rides the load
                # shadow on Q7.  One frame to the SEngine partner (XOR 1).
                par = _rep % 2
                rd = [None] * NCORES
                rd[1] = (0, 1)
                # queue 1: untriggered preps must not stall the bulk loads
                # (queue 0) — SDMA drains each ring FIFO in order
                prep = nc.gpsimd.remote_dma_broadcast(
                    uT_recv[:, par, :], uT_send[:, par, :],
                    remote_sem=rsem, local_sem=lsem, rdests=rd, queue_num=1)

                # -------- partial Xsum^T (TensorE, lands [c-part, (ck b)]) --
                # lhsT = X tile (stationary): out[c, 0] = sum_n X[n, c];
                # the AllReduce then runs in transposed layout, so no
                # pre-bounce two-stage drain and no post-land transposes
                xs2_ps = psum.tile([128, JT * B], f32, tag="small", bufs=2)
                for b in range(B):
                    for ck in range(JT):
                        col = ck * B + b
                        for ch in range(CH):
                            nc.tensor.matmul(
                                xs2_ps[:, col : col + 1],
                                x_bf[:, b * CH + ch, ck * 128 : (ck + 1) * 128],
                                ones_red[:],
                                start=(ch == 0),
                                stop=(ch == CH - 1),
                            )
                nc.vector.tensor_copy(xspT_sb[:], xs2_ps[:])

                # ---------------- AllReduce of [B, C] partial Xsum ----------
                ar_in = dram.tile([128, JT * B], f32, name=f"ar_in{_rep}")
                ar_out = dram.tile(
                    [128, JT * B], f32, addr_space="Shared", name=f"ar_out{_rep}"
                )
                bounce = nc.scalar.dma_start(ar_in[:], xspT_sb[:])
                if use_collective:
                    nc.gpsimd.collective_compute(
                        "AllReduce",
                        add,
                        replica_groups=[list(range(NCORES))],
                        ins=[ar_in.opt()],
                        outs=[ar_out.opt()],
                    )
                else:
                    nc.scalar.dma_start(ar_out[:], ar_in[:])
                nc.scalar.dma_start(xsumTf[:], ar_out[:])
                nc.scalar.copy(
                    xsumT_sb[:].rearrange("p ck b -> p (ck b)"), xsumTf[:]
                )

                # ---------------- Wk_h^T via TensorE transpose --------------
                for ck in range(JT):
                    for jt in range(JH):
                        wt_ps = psum.tile(
                            [128, 128], bf16, tag="tr", bufs=2, name=f"wt{jt}_{ck}"
                        )
                        nc.tensor.transpose(
                            wt_ps[:],
                            wk_sb[:, jt, ck * 128 : (ck + 1) * 128],
                            ident_bf[:],
                        )
                        nc.vector.tensor_copy(
                            wkT_sb[:, ck, jt * 128 : (jt + 1) * 128], wt_ps[:]
                        )

                # HAM warm-up: keep the PE clock gate at full rate through the
                # transpose-heavy AllReduce window.
                warm_ps = psum.tile([1, 512], f32, tag="small", bufs=2)
                N_WARM = 12
                for i in range(N_WARM):
                    nc.tensor.matmul(
                        warm_ps[:],
                        ones_red[:],
                        wk_sb[:, i % JH, 0:512],
                        start=(i == 0),
                        stop=(i == N_WARM - 1),
                    )

                # ---------------- X^T via TensorE transpose ----------------
                for b in range(B):
                    for ch in range(CH):
                        col = (b * CH + ch) * 128
                        for ck in range(JT):
                            xt_ps2 = psum.tile(
                                [128, 128], bf16, tag="tr", bufs=2,
                                name=f"xtr{b}_{ch}_{ck}",
                            )
                            nc.tensor.transpose(
                                xt_ps2[:],
                                x_bf[:, b * CH + ch, ck * 128 : (ck + 1) * 128],
                                ident_bf[:],
                            )
                            if ck % 2 == 0:
                                nc.scalar.copy(
                                    xT_sb[:, ck, col : col + 128], xt_ps2[:]
                                )
                            else:
                                nc.vector.tensor_copy(
                                    xT_sb[:, ck, col : col + 128], xt_ps2[:]
                                )

                # ------- t_h[j, b] = sum_c Wk_h^T[c, j] Xsum^T[c, b] --------
                # lhsT = Wk_h^T 128-col slices (stationary), rhs = Xsum^T:
                # lands t directly with j on partitions — no drain/transpose
                for js in range(JH):
                    t_ps = psum.tile(
                        [128, 2], f32, tag="tr", bufs=2, name=f"td{js}"
                    )
                    for ck in range(JT):
                        nc.tensor.matmul(
                            t_ps[:],
                            wkT_sb[:, ck, js * 128 : (js + 1) * 128],
                            xsumT_sb[:, ck, :],
                            start=(ck == 0),
                            stop=(ck == JT - 1),
                        )
                    nc.vector.tensor_copy(t_sb[:, js, :], t_ps[:])

                # ------ u_h^T[c', b] = sum_j Wq_h[j, c'] t_h[j, b] ----------
                # lhsT = Wq_h 128-col slices (stationary), rhs = t_h: lands
                # u^T chunks straight into the send buffer — no drain and no
                # transposes between u and the exchange trigger
                tcopies = []
                for ck in range(JT):
                    u_ps = psum.tile(
                        [128, 2], f32, tag="tr", bufs=2, name=f"ud{ck}"
                    )
                    for jt in range(JH):
                        nc.tensor.matmul(
                            u_ps[:],
                            wq_sb[:, jt, ck * 128 : (ck + 1) * 128],
                            t_sb[:, jt, :],
                            start=(jt == 0),
                            stop=(jt == JH - 1),
                        )
                    cp = nc.vector.tensor_copy(
                        uT_send[:, par, ck * B : (ck + 1) * B], u_ps[:]
                    )
                    tcopies.append(cp)

                # fire the pair exchange once the send buffer is complete
                trig = nc.gpsimd.trigger_dma(count=1, queue_num=1)
                for cp in tcopies:
                    tile.add_dep_helper(trig.ins, cp.ins, sync=True,
                                        reason="u^T drained before trigger")

                # my half casts to bf16 immediately; the partner's cast
                # carries the attached remote-sem wait, so the first six
                # final matmuls (mine) overlap the partner's arrival
                cs_mine = nc.vector.tensor_copy(uTs_bf[:], uT_send[:, par, :])
                cs_part = nc.vector.tensor_copy(uTr_bf[:], uT_recv[:, par, :])
                tile.add_dep_helper(cs_part.ins, trig.ins, sync=False,
                                    reason="partner cast after trigger")
                # the attached rsem wait blocks the DVE stream; every DVE op
                # the AllReduce bounce needs must be scheduled before it,
                # else the cores deadlock through the collective
                tile.add_dep_helper(cs_part.ins, bounce.ins, sync=False,
                                    reason="rsem wait after the CC bounce")
                inject_specs.append(
                    ("vector", rsem, 2 * (_rep + 1), cs_part.ins))

                # ------- scores^T = scale * diag(u^T . X^T) -----------------
                sc_ps = psum.tile([B, 512], f32, tag="mid", bufs=1, name="sc")
                for half, ubuf in ((0, uTs_bf), (1, uTr_bf)):
                    for ck in range(JT):
                        nc.tensor.matmul(
                            sc_ps[:],
                            ubuf[:, ck * B : (ck + 1) * B],
                            xT_sb[:, ck, :],
                            start=(half == 0 and ck == 0),
                            stop=(half == 1 and ck == JT - 1),
                        )
                nc.scalar.mul(out_row2[:, 0:NS], sc_ps[:, 0:NS], SCALE)
                nc.sync.dma_start(out_d[0:1, :], out_row2[0:1, 0:NS])
                nc.vector.tensor_scalar_mul(
                    out_row2[:, NS : 2 * NS], sc_ps[:, NS : 2 * NS], SCALE
                )
                nc.sync.dma_start(out_d[1:2, :], out_row2[1:2, NS : 2 * NS])

    # HW-only cross-core waits, invisible to the scheduling sim
    for eng_name, sem, val, target in inject_specs:
        _attach_wait(target, sem, val)
    # end-of-NEFF quiesce: all sends flushed, all arrivals seen
    nc.gpsimd.wait_ge(lsem, 16 * repeats)
    nc.gpsimd.wait_ge(rsem, 2 * repeats)

    nc.compile()
    return nc


def _get_nc():
    global _compiled_nc
    if _compiled_nc is None:
        _compiled_nc = _build_and_compile()
    return _compiled_nc


def make_in_maps(X, W_qkv):
    import ml_dtypes

    X = np.asarray(X, dtype=np.float32).astype(ml_dtypes.bfloat16)
    W = np.asarray(W_qkv, dtype=np.float32).astype(ml_dtypes.bfloat16)
    assert X.shape == (B, N, C) and W.shape == (2 * C, C)
    halves = []
    for h in range(2):
        wq_h = W[h * 384 : (h + 1) * 384, :]
        wk_h = W[C + h * 384 : C + (h + 1) * 384, :]
        halves.append(np.ascontiguousarray(np.concatenate([wq_h, wk_h], axis=0)))
    return [
        {
            "x_in": np.ascontiguousarray(X[:, i * NS : (i + 1) * NS, :]),
            "w_in": halves[i % 2],
        }
        for i in range(NCORES)
    ]


def assemble_out(results):
    return np.concatenate(
        [results[i]["scores_out"] for i in range(NCORES)], axis=1
    ).astype(np.float32)


def kernel(X, W_qkv):
    from concourse import bass_utils

    nc = _get_nc()
    res = bass_utils.run_bass_kernel_spmd(
        nc, make_in_maps(X, W_qkv), core_ids=list(range(NCORES))
    )
    return assemble_out(res.results)



# revision 2
# speedup vs baseline: 1.0068x; 1.0068x over previous
"""Distributed TRN2 kernel for nn_Att_scores (attention score double-sum).

Math: qkv = X @ W_qkv.T, scores = (q k^T * scale).sum(heads).sum(keys)
collapses exactly to
    Xsum[b]      = sum_n X[b, n, :]                      # [C]
    u[b]         = Wq^T (Wk Xsum[b])                     # [C]
    scores[b, n] = scale * X[b, n, :] . u[b]

Distribution (8 cores): X row-sharded (256 rows/batch per core), W split
8-way by qkv rows (core i holds rows i*96:(i+1)*96 of Wq and Wk), so
u = sum_i Wq_i^T (Wk_i Xsum).  Host prep is layout/dtype only: bf16 cast,
X^T packed [c-part, ck, (b n)], Wk pre-transposed — no PE transposes, no
ones-matmul reductions on device.

The two 6KB cross-core reductions (partial Xsum, partial u) are
single-frame SWDGE mesh broadcasts (collective_compute's 8-core AllReduce
has a ~10us ncfw latency floor): under tc.Switch on partition_id each
core broadcasts its [128, 12] f32 partial to slot my_id on ALL 8 cores
(self included via the (0,0) loopback dest), then a 3-op strided tree add
on DVE folds the 8 slots.  One ~1us Q7 descgen prep (hidden in the load
shadow) + one trigger per exchange, on separate SWDGE queues so each
trigger inherits only its own prep's deferred read of the send buffer.

The local n-reduction runs as a bf16 tensor_add tree (DVE 2x mode) plus a
short TensorReduce tail — InstTensorReduce itself has no fast mode.

Pipelining: NPAR=4 exchange parities with PER-PARITY remote/local sems,
input tiles rotate via tag pools (bufs=XBUFS).  Wait values are the
maximum attainable count on that sem at that point, which makes them
race-free under the per-lane sem increment model:
  * send-buffer reuse (lsx/lsu): at the rep-r writer, parity-p frames in
    existence are exactly reps p, p+4, ..., r-4 (frame r's trigger is
    gated on this very writer), so lsx[p] == 16*(r//4) is exact.
  * arrival folds (rsx/rsu): skew across cores is bounded by the x_sb
    reuse chain (a core can only fire frame r+4 after every core's
    frame r+1 delivered everywhere), so parity-p arrivals at the rep-r
    fold are exactly reps p..r of all 8 senders: 16*(r//4 + 1).
Cross-core arrivals are enforced by HW-only waits attached
post-scheduling (invisible to Tile, like the baseline's _attach_wait).
"""

import numpy as np

B = 2
N = 2048
C = 768
H = 12
HD = C // H
SCALE = float(HD) ** -0.5
NCORES = 8
NS = N // NCORES          # 256 rows of each batch per core
JT = C // 128             # 6 128-row chunks of C
WS = C // NCORES          # 96 rows of Wq / Wk per core
FB = B * NS               # 512 = free dim of X^T per core
NPAR = 4                  # exchange-buffer parities
XBUFS = 3                 # x_sb rotation depth

_compiled_nc = None


def _attach_wait(target_ins, sem, val):
    """Attach a HW-only semaphore wait directly to target_ins's dispatch
    conditions, post-scheduling (invisible to the Tile sim; a standalone
    injected EventSemaphore would get merged by the legalizer)."""
    import concourse.mybir as mb

    w = mb.SyncWait(
        sync_type="semaphore",
        id=sem.num,
        ant_name=sem.name,
        wait_mode="sem-ge-imm",
        wait_value=val,
    )
    si = target_ins.sync_info
    if si is None:
        target_ins.sync_info = mb.SyncInfo(on_wait=[w], on_update=[])
    else:
        si.on_wait.append(w)


def _build_and_compile(use_collective=True, repeats=1):
    import concourse.bass as bass  # noqa: F401
    import concourse.bacc as bacc
    import concourse.tile as tile
    import concourse.mybir as mybir

    f32 = mybir.dt.float32
    bf16 = mybir.dt.bfloat16
    add = mybir.AluOpType.add
    nc = bacc.Bacc(
        "TRN2",
        target_bir_lowering=False,
        debug=False,
        num_devices=NCORES,
        num_swdge_queues=3,
    )

    # X^T shard, host-packed [128, JT, FB] (c-part, ck, (b n)), bf16
    x_d = nc.dram_tensor("x_in", [128, JT * FB], bf16, kind="ExternalInput")
    # Wq_i rows [96, 768] natural (lhsT for the u matmul)
    wq_d = nc.dram_tensor("wq_in", [WS, C], bf16, kind="ExternalInput")
    # Wk_i^T host-packed [128, JT, 96] (c-part, ck, j) (lhsT for t matmul)
    wkT_d = nc.dram_tensor("wkT_in", [128, JT * WS], bf16, kind="ExternalInput")
    out_d = nc.dram_tensor("scores_out", [B, NS], f32, kind="ExternalOutput")

    rsx = [nc.alloc_semaphore(f"rsx{p}") for p in range(NPAR)]
    rsu = [nc.alloc_semaphore(f"rsu{p}") for p in range(NPAR)]
    lsx = [nc.alloc_semaphore(f"lsx{p}") for p in range(NPAR)]
    lsu = [nc.alloc_semaphore(f"lsu{p}") for p in range(NPAR)]

    ALLDESTS = [(0, k) for k in range(NCORES)]
    inject_specs = []  # (sem, val, target_ins)

    with tile.TileContext(nc) as tc:
        with (
            tc.tile_pool(name="sbuf", bufs=1) as pool,
            tc.tile_pool(name="psum", bufs=1, space="PSUM") as psum,
        ):
            xsend = pool.tile([128, NPAR, JT * B], f32)
            exch_x = pool.tile([128, NPAR, NCORES, JT * B], f32)
            xtree = pool.tile([128, 6, JT * B], f32)
            usend = pool.tile([128, NPAR, JT * B], f32)
            exch_u = pool.tile([128, NPAR, NCORES, JT * B], f32)
            utree = pool.tile([128, 6, JT * B], f32)

            pid = nc.gpsimd.partition_id()

            for _rep in range(repeats):
                par = _rep % NPAR

                x_sb = pool.tile([128, JT, FB], bf16, tag="x", bufs=XBUFS)
                wq_sb = pool.tile([WS, JT, 128], bf16, tag="wq", bufs=2)
                wkT_sb = pool.tile([128, JT, WS], bf16, tag="wkT", bufs=2)
                xtr1 = pool.tile([128, JT, B, 128], bf16, tag="xt1", bufs=2)
                xtr2 = pool.tile([128, JT, B, 64], bf16, tag="xt2", bufs=2)
                xtr3 = pool.tile([128, JT, B, 32], bf16, tag="xt3", bufs=2)
                xsumT_bf = pool.tile([128, JT, B], bf16, tag="xs", bufs=2)
                t_bf = pool.tile([WS, B], bf16, tag="tb", bufs=2)
                uT_bf = pool.tile([128, JT, B], bf16, tag="ub", bufs=2)
                outa = pool.tile([B, NS], f32, tag="oa", bufs=2)
                outb = pool.tile([B, NS], f32, tag="ob", bufs=2)

                # ---- exchange preps (descgen rides the load shadow) ----
                for case in tc.Switch(pid, NCORES):
                    nc.gpsimd.remote_dma_broadcast(
                        exch_x[:, par, case, :], xsend[:, par, :],
                        remote_sem=rsx[par], local_sem=lsx[par],
                        rdests=ALLDESTS, queue_num=1,
                    )
                    nc.gpsimd.remote_dma_broadcast(
                        exch_u[:, par, case, :], usend[:, par, :],
                        remote_sem=rsu[par], local_sem=lsu[par],
                        rdests=ALLDESTS, queue_num=2,
                    )

                # ---- loads, split across the two HWDGE rings ----------
                nc.sync.dma_start(x_sb[:, 0:3, :], x_d[:, 0 : 3 * FB])
                nc.scalar.dma_start(x_sb[:, 3:6, :], x_d[:, 3 * FB : 6 * FB])
                nc.sync.dma_start(wkT_sb[:], wkT_d[:])
                nc.sync.dma_start(wq_sb[:], wq_d[:])

                # ---- partial Xsum^T: bf16 add-tree + short reduce,
                # split per DMA half so DVE starts as each half lands ----
                xv = x_sb[:].rearrange("p t (b f) -> p t b f", b=B)
                reds = []
                for h in (0, 1):
                    s = slice(3 * h, 3 * (h + 1))
                    nc.vector.tensor_add(
                        xtr1[:, s], xv[:, s, :, 0:128], xv[:, s, :, 128:256]
                    )
                    nc.vector.tensor_add(
                        xtr2[:, s], xtr1[:, s, :, 0:64], xtr1[:, s, :, 64:128]
                    )
                    nc.vector.tensor_add(
                        xtr3[:, s], xtr2[:, s, :, 0:32], xtr2[:, s, :, 32:64]
                    )
                    red = nc.vector.tensor_reduce(
                        xsend[:, par, 6 * h : 6 * (h + 1)].rearrange(
                            "p (t b) -> p t b", b=B
                        ),
                        xtr3[:, s],
                        axis=mybir.AxisListType.X,
                        op=add,
                    )
                    reds.append(red)
                if _rep >= NPAR:
                    # parity-p frames in existence: reps p..rep-NPAR only
                    for red in reds:
                        inject_specs.append(
                            (lsx[par], 16 * (_rep // NPAR), red.ins))

                # ---- fire the x-exchange ------------------------------
                trig_x = nc.gpsimd.trigger_dma(count=1, queue_num=1)
                for red in reds:
                    tile.add_dep_helper(trig_x.ins, red.ins, sync=True,
                                        reason="Xsum partial before x trigger")

                # ---- fold the 8 partials (DVE, program order) ---------
                a1 = nc.vector.tensor_add(
                    xtree[:, 0:4, :], exch_x[:, par, 0:4, :], exch_x[:, par, 4:8, :]
                )
                tile.add_dep_helper(a1.ins, trig_x.ins, sync=False,
                                    reason="x fold after x trigger")
                inject_specs.append((rsx[par], 16 * (_rep // NPAR + 1), a1.ins))
                nc.vector.tensor_add(
                    xtree[:, 4:6, :], xtree[:, 0:2, :], xtree[:, 2:4, :]
                )
                nc.vector.tensor_add(
                    xsumT_bf[:].rearrange("p t b -> p (t b)"),
                    xtree[:, 4, :], xtree[:, 5, :],
                )

                # ---- t_i[j, b] = sum_c Wk_i^T[c, j] Xsum^T[c, b] ------
                t_ps = psum.tile([WS, B], f32, tag="t", bufs=2)
                for ck in range(JT):
                    nc.tensor.matmul(
                        t_ps[:],
                        wkT_sb[:, ck, :],
                        xsumT_bf[:, ck, :],
                        start=(ck == 0),
                        stop=(ck == JT - 1),
                    )
                nc.scalar.copy(t_bf[:], t_ps[:])

                # ---- u_i^T[c', b] = sum_j Wq_i[j, c'] t_i[j, b] -------
                u_ps = psum.tile([128, JT * B], f32, tag="u", bufs=2)
                for ck in range(JT):
                    nc.tensor.matmul(
                        u_ps[:, ck * B : (ck + 1) * B],
                        wq_sb[:, ck, :],
                        t_bf[:],
                        start=True,
                        stop=True,
                    )
                udrain = nc.scalar.copy(usend[:, par, :], u_ps[:])
                if _rep >= NPAR:
                    inject_specs.append((lsu[par], 16 * (_rep // NPAR), udrain.ins))

                # ---- fire the u-exchange ------------------------------
                trig_u = nc.gpsimd.trigger_dma(count=1, queue_num=2)
                tile.add_dep_helper(trig_u.ins, udrain.ins, sync=True,
                                    reason="u drained before u trigger")

                b1 = nc.vector.tensor_add(
                    utree[:, 0:4, :], exch_u[:, par, 0:4, :], exch_u[:, par, 4:8, :]
                )
                tile.add_dep_helper(b1.ins, trig_u.ins, sync=False,
                                    reason="u fold after u trigger")
                inject_specs.append((rsu[par], 16 * (_rep // NPAR + 1), b1.ins))
                nc.vector.tensor_add(
                    utree[:, 4:6, :], utree[:, 0:2, :], utree[:, 2:4, :]
                )
                nc.vector.tensor_add(
                    uT_bf[:].rearrange("p t b -> p (t b)"),
                    utree[:, 4, :], utree[:, 5, :],
                )

                # ---- scores: [2, 512] diagonal-block accumulation -----
                sc_ps = psum.tile([B, FB], f32, tag="sc", bufs=2)
                for ck in range(JT):
                    nc.tensor.matmul(
                        sc_ps[:],
                        uT_bf[:, ck, :],
                        x_sb[:, ck, :],
                        start=(ck == 0),
                        stop=(ck == JT - 1),
                    )
                nc.scalar.mul(outa[:], sc_ps[:, 0:NS], SCALE)
                nc.sync.dma_start(out_d[0:1, :], outa[0:1, :])
                nc.scalar.mul(outb[:], sc_ps[:, NS : 2 * NS], SCALE)
                nc.scalar.dma_start(out_d[1:2, :], outb[1:2, :])

    # HW-only cross-core waits, invisible to the scheduling sim
    for sem, val, target in inject_specs:
        _attach_wait(target, sem, val)
    # end-of-NEFF quiesce: all sends flushed, all arrivals seen
    for p in range(NPAR):
        cnt = (repeats - p + NPAR - 1) // NPAR if repeats > p else 0
        if cnt > 0:
            nc.gpsimd.wait_ge(lsx[p], 16 * cnt)
            nc.gpsimd.wait_ge(lsu[p], 16 * cnt)
            nc.gpsimd.wait_ge(rsx[p], 16 * cnt)
            nc.gpsimd.wait_ge(rsu[p], 16 * cnt)

    nc.compile()
    return nc


def _get_nc():
    global _compiled_nc
    if _compiled_nc is None:
        _compiled_nc = _build_and_compile()
    return _compiled_nc


def make_in_maps(X, W_qkv):
    import ml_dtypes

    bf = ml_dtypes.bfloat16
    X = np.asarray(X, dtype=np.float32)
    W = np.asarray(W_qkv, dtype=np.float32)
    assert X.shape == (B, N, C) and W.shape == (2 * C, C)
    maps = []
    for i in range(NCORES):
        xs = X[:, i * NS : (i + 1) * NS, :]              # [B, NS, C]
        xt = xs.transpose(2, 0, 1).reshape(C, FB)        # [c, (b n)]
        xt = xt.reshape(JT, 128, FB).transpose(1, 0, 2)  # [128, ck, (b n)]
        wq = W[i * WS : (i + 1) * WS, :]                 # [96, 768]
        wkT = W[C + i * WS : C + (i + 1) * WS, :].T      # [768, 96]
        wkT = wkT.reshape(JT, 128, WS).transpose(1, 0, 2)
        maps.append(
            {
                "x_in": np.ascontiguousarray(xt.reshape(128, JT * FB)).astype(bf),
                "wq_in": np.ascontiguousarray(wq).astype(bf),
                "wkT_in": np.ascontiguousarray(wkT.reshape(128, JT * WS)).astype(bf),
            }
        )
    return maps


def assemble_out(results):
    return np.concatenate(
        [results[i]["scores_out"] for i in range(NCORES)], axis=1
    ).astype(np.float32)


def kernel(X, W_qkv):
    from concourse import bass_utils

    nc = _get_nc()
    res = bass_utils.run_bass_kernel_spmd(
        nc, make_in_maps(X, W_qkv), core_ids=list(range(NCORES))
    )
    return assemble_out(res.results)
